# revision 2
# baseline (speedup 1.0000x reference)
"""Batched QK^T matmul on 8 Trainium2 NeuronCores.

Problem: mat_0 [8, 2048, 1024] f32, mat_1 [8, 2048, 1024] f32
         out   [8, 2048, 2048] f32 = einsum('bne,bme->bnm')

Sharding: data-parallel over batch — core i computes batch element i:
         C = A @ B^T with A = mat_0[i], B = mat_1[i].

Per-core kernel structure (Tile framework):
  - Both operands need the contraction dim (e) on SBUF partitions, so both
    are transposed on-chip with PE transpose-mode matmuls (fp32 DMA
    transpose does not exist on TRN2).
  - B^T is built once and cached in SBUF ([128, 8, 2048] = 64KB/partition).
  - For each 128-row block of A: transpose to A^T chunks, then accumulate
    over the 8 e-chunks into [128, 512] PSUM tiles, copy back and DMA out.
"""

import sys

if "/opt/trn_rl_repo" not in sys.path:
    sys.path.insert(0, "/opt/trn_rl_repo")

import numpy as np

import concourse.bass as bass  # noqa: E402
import concourse.mybir as mybir  # noqa: E402
import concourse.tile as tile  # noqa: E402
from concourse import bacc  # noqa: E402
from concourse.bass_utils import run_bass_kernel_spmd  # noqa: E402
from concourse.masks import make_identity  # noqa: E402

P = 128

# Hardcoded problem shape (per spec nn_AttentionMatrix_41841571398230)
B_FULL, N_FULL, M_FULL, E_FULL = 8, 2048, 2048, 1024


def qkt_kernel(tc, a, b, c, n, m, e, mm_mode="f32", mg=512):
    """Emit per-core C[n, m] = A[n, :] @ B[m, :]^T.

    mm_mode: 'f32' | 'f32r' | 'fp16' | 'bf16'
      f32   — everything fp32 (4 cyc/row matmul, exact)
      f32r  — fp32 data, matmuls run with float32r bitcast (1 cyc/row @ N>=256)
      fp16  — transpose copyback casts to fp16; matmuls fp16 (1 cyc/row)
      bf16  — same with bf16
    """
    nc = tc.nc
    f32 = mybir.dt.float32
    op_dtype = {
        "f32": f32,
        "f32r": f32,
        "fp16": mybir.dt.float16,
        "bf16": mybir.dt.bfloat16,
    }[mm_mode]
    mg = min(mg, m)
    n_blocks = n // P
    m_blocks = m // P
    e_chunks = e // P
    m_groups = m // mg

    with (
        tc.tile_pool(name="const", bufs=1) as const_pool,
        tc.tile_pool(name="stage", bufs=3) as stage_pool,
        tc.tile_pool(name="tpsum", bufs=2, space="PSUM") as tpsum_pool,
        tc.tile_pool(name="btp", bufs=1) as bt_pool,
        tc.tile_pool(name="atp", bufs=2) as at_pool,
        tc.tile_pool(name="mpsum", bufs=4, space="PSUM") as mpsum_pool,
        tc.tile_pool(name="co", bufs=3) as co_pool,
    ):
        ident = const_pool.tile([P, P], f32)
        make_identity(nc, ident)

        # ---- Build B^T cached in SBUF: bt[p, k, m] = B[m, k*128+p] ----
        bt = bt_pool.tile([P, e_chunks, m], op_dtype)
        for mb in range(m_blocks):
            stage = stage_pool.tile([P, e], f32, tag="stage")
            nc.sync.dma_start(stage, b[mb * P : (mb + 1) * P, :])
            for k in range(e_chunks):
                pt = tpsum_pool.tile([P, P], f32, tag="tp")
                nc.tensor.transpose(pt, stage[:, k * P : (k + 1) * P], ident)
                nc.any.tensor_copy(out=bt[:, k, mb * P : (mb + 1) * P], in_=pt)

        # ---- Main loop over 128-row blocks of A ----
        for nb in range(n_blocks):
            stage = stage_pool.tile([P, e], f32, tag="stage")
            nc.sync.dma_start(stage, a[nb * P : (nb + 1) * P, :])
            at = at_pool.tile([P, e_chunks, P], op_dtype, tag="at")
            for k in range(e_chunks):
                pt = tpsum_pool.tile([P, P], f32, tag="tp")
                nc.tensor.transpose(pt, stage[:, k * P : (k + 1) * P], ident)
                nc.any.tensor_copy(out=at[:, k, :], in_=pt)

            for g in range(m_groups):
                ps = mpsum_pool.tile([P, mg], f32, tag="ps")
                for k in range(e_chunks):
                    lhsT = at[:, k, :]
                    rhs = bt[:, k, g * mg : (g + 1) * mg]
                    if mm_mode == "f32r":
                        lhsT = lhsT.bitcast(mybir.dt.float32r)
                        rhs = rhs.bitcast(mybir.dt.float32r)
                    nc.tensor.matmul(
                        ps,
                        lhsT,
                        rhs,
                        start=(k == 0),
                        stop=(k == e_chunks - 1),
                    )
                ot = co_pool.tile([P, mg], f32, tag="ot")
                nc.any.tensor_copy(out=ot, in_=ps)
                nc.sync.dma_start(c[nb * P : (nb + 1) * P, g * mg : (g + 1) * mg], ot)


def build_qkt(n, m, e, mm_mode="f32", mg=512):
    nc = bacc.Bacc(None, target_bir_lowering=False)
    with tile.TileContext(nc) as tc:
        with tc.tile_pool(name="dram", bufs=1, space="DRAM") as dram:
            a = dram.tile([n, e], mybir.dt.float32, kind="ExternalInput", name="mat_0")
            b = dram.tile([m, e], mybir.dt.float32, kind="ExternalInput", name="mat_1")
            c = dram.tile([n, m], mybir.dt.float32, kind="ExternalOutput", name="out")
            qkt_kernel(tc, a[:], b[:], c[:], n, m, e, mm_mode=mm_mode, mg=mg)
    nc.compile()
    return nc, a.name, b.name, c.name


_CACHE = {}


def _get_built(n, m, e, mm_mode, mg=512):
    key = (n, m, e, mm_mode, mg)
    if key not in _CACHE:
        _CACHE[key] = build_qkt(n, m, e, mm_mode=mm_mode, mg=mg)
    return _CACHE[key]


def run_qkt(mat_0, mat_1, mm_mode="f32", mg=512, trace=False):
    """Run the sharded kernel on full inputs [b, n, e], [b, m, e]."""
    bsz, n, e = mat_0.shape
    _, m, _ = mat_1.shape
    nc, a_name, b_name, c_name = _get_built(n, m, e, mm_mode, mg)
    in_maps = [
        {
            a_name: np.ascontiguousarray(mat_0[i], dtype=np.float32),
            b_name: np.ascontiguousarray(mat_1[i], dtype=np.float32),
        }
        for i in range(bsz)
    ]
    res = run_bass_kernel_spmd(nc, in_maps, core_ids=list(range(bsz)), trace=trace)
    out = np.stack([res.results[i][c_name] for i in range(bsz)], axis=0)
    return out, res


def kernel(mat_0, mat_1):
    out, _ = run_qkt(
        np.asarray(mat_0, dtype=np.float32),
        np.asarray(mat_1, dtype=np.float32),
        mm_mode="f32",
    )
    return out


# revision 4
# speedup vs baseline: 2.3812x; 2.3812x over previous
"""Batched QK^T matmul on 8 Trainium2 NeuronCores.

Problem: mat_0 [8, 2048, 1024] f32, mat_1 [8, 2048, 1024] f32
         out   [8, 2048, 2048] f32 = einsum('bne,bme->bnm')

Sharding: data-parallel over batch — core i computes batch element i:
         C = A @ B^T with A = mat_0[i], B = mat_1[i].

Per-core kernel structure (Tile framework):
  - Both operands need the contraction dim (e) on SBUF partitions, so both
    are transposed on-chip with PE transpose-mode matmuls (fp32 DMA
    transpose does not exist on TRN2).
  - B^T is built once and cached in SBUF ([128, 8, 2048] = 64KB/partition).
  - For each 128-row block of A: transpose to A^T chunks, then accumulate
    over the 8 e-chunks into [128, 512] PSUM tiles, copy back and DMA out.
"""

import sys

if "/opt/trn_rl_repo" not in sys.path:
    sys.path.insert(0, "/opt/trn_rl_repo")

import numpy as np

import concourse.bass as bass  # noqa: E402
import concourse.mybir as mybir  # noqa: E402
import concourse.tile as tile  # noqa: E402
from concourse import bacc  # noqa: E402
from concourse.bass_utils import run_bass_kernel_spmd  # noqa: E402
from concourse.masks import make_identity  # noqa: E402

P = 128

# Hardcoded problem shape (per spec nn_AttentionMatrix_41841571398230)
B_FULL, N_FULL, M_FULL, E_FULL = 8, 2048, 2048, 1024


def qkt_kernel(tc, a, b, c, n, m, e, mm_mode="f32", mg=512):
    """Emit per-core C[n, m] = A[n, :] @ B[m, :]^T.

    mm_mode: 'f32' | 'f32r' | 'fp16' | 'bf16'
      f32   — everything fp32 (4 cyc/row matmul, exact)
      f32r  — fp32 data, matmuls run with float32r bitcast (1 cyc/row @ N>=256)
      fp16  — transpose copyback casts to fp16; matmuls fp16 (1 cyc/row)
      bf16  — same with bf16
    """
    nc = tc.nc
    f32 = mybir.dt.float32
    op_dtype = {
        "f32": f32,
        "f32r": mybir.dt.float32r,
        "fp16": mybir.dt.float16,
        "bf16": mybir.dt.bfloat16,
    }[mm_mode]
    mg = min(mg, m)
    n_blocks = n // P
    m_blocks = m // P
    e_chunks = e // P
    m_groups = m // mg

    with (
        tc.tile_pool(name="const", bufs=1) as const_pool,
        tc.tile_pool(name="stage", bufs=3) as stage_pool,
        tc.tile_pool(name="tpsum", bufs=2, space="PSUM") as tpsum_pool,
        tc.tile_pool(name="btp", bufs=1) as bt_pool,
        tc.tile_pool(name="atp", bufs=2) as at_pool,
        tc.tile_pool(name="mpsum", bufs=4, space="PSUM") as mpsum_pool,
        tc.tile_pool(name="co", bufs=3) as co_pool,
    ):
        ident = const_pool.tile([P, P], f32)
        make_identity(nc, ident)

        # ---- Build B^T cached in SBUF: bt[p, k, m] = B[m, k*128+p] ----
        bt = bt_pool.tile([P, e_chunks, m], op_dtype)
        for mb in range(m_blocks):
            stage = stage_pool.tile([P, e], f32, tag="stage")
            nc.sync.dma_start(stage, b[mb * P : (mb + 1) * P, :])
            for k in range(e_chunks):
                pt = tpsum_pool.tile([P, P], f32, tag="tp")
                nc.tensor.transpose(pt, stage[:, k * P : (k + 1) * P], ident)
                nc.any.tensor_copy(out=bt[:, k, mb * P : (mb + 1) * P], in_=pt)

        # ---- Main loop over 128-row blocks of A ----
        for nb in range(n_blocks):
            stage = stage_pool.tile([P, e], f32, tag="stage")
            nc.sync.dma_start(stage, a[nb * P : (nb + 1) * P, :])
            at = at_pool.tile([P, e_chunks, P], op_dtype, tag="at")
            for k in range(e_chunks):
                pt = tpsum_pool.tile([P, P], f32, tag="tp")
                nc.tensor.transpose(pt, stage[:, k * P : (k + 1) * P], ident)
                nc.any.tensor_copy(out=at[:, k, :], in_=pt)

            for g in range(m_groups):
                ps = mpsum_pool.tile([P, mg], f32, tag="ps")
                for k in range(e_chunks):
                    nc.tensor.matmul(
                        ps,
                        at[:, k, :],
                        bt[:, k, g * mg : (g + 1) * mg],
                        start=(k == 0),
                        stop=(k == e_chunks - 1),
                    )
                ot = co_pool.tile([P, mg], f32, tag="ot")
                nc.any.tensor_copy(out=ot, in_=ps)
                nc.sync.dma_start(c[nb * P : (nb + 1) * P, g * mg : (g + 1) * mg], ot)


def build_qkt(n, m, e, mm_mode="f32", mg=512):
    nc = bacc.Bacc(None, target_bir_lowering=False)
    with tile.TileContext(nc) as tc:
        with tc.tile_pool(name="dram", bufs=1, space="DRAM") as dram:
            a = dram.tile([n, e], mybir.dt.float32, kind="ExternalInput", name="mat_0")
            b = dram.tile([m, e], mybir.dt.float32, kind="ExternalInput", name="mat_1")
            c = dram.tile([n, m], mybir.dt.float32, kind="ExternalOutput", name="out")
            qkt_kernel(tc, a[:], b[:], c[:], n, m, e, mm_mode=mm_mode, mg=mg)
    nc.compile()
    return nc, a.name, b.name, c.name


_CACHE = {}


def _get_built(n, m, e, mm_mode, mg=512):
    key = (n, m, e, mm_mode, mg)
    if key not in _CACHE:
        _CACHE[key] = build_qkt(n, m, e, mm_mode=mm_mode, mg=mg)
    return _CACHE[key]


def run_qkt(mat_0, mat_1, mm_mode="f32", mg=512, trace=False):
    """Run the sharded kernel on full inputs [b, n, e], [b, m, e]."""
    bsz, n, e = mat_0.shape
    _, m, _ = mat_1.shape
    nc, a_name, b_name, c_name = _get_built(n, m, e, mm_mode, mg)
    in_maps = [
        {
            a_name: np.ascontiguousarray(mat_0[i], dtype=np.float32),
            b_name: np.ascontiguousarray(mat_1[i], dtype=np.float32),
        }
        for i in range(bsz)
    ]
    res = run_bass_kernel_spmd(nc, in_maps, core_ids=list(range(bsz)), trace=trace)
    out = np.stack([res.results[i][c_name] for i in range(bsz)], axis=0)
    return out, res


def kernel(mat_0, mat_1):
    out, _ = run_qkt(
        np.asarray(mat_0, dtype=np.float32),
        np.asarray(mat_1, dtype=np.float32),
        mm_mode="f32",
    )
    return out


# revision 7
# speedup vs baseline: 2.3906x; 1.0039x over previous
"""Batched QK^T matmul on 8 Trainium2 NeuronCores.

Problem: mat_0 [8, 2048, 1024] f32, mat_1 [8, 2048, 1024] f32
         out   [8, 2048, 2048] f32 = einsum('bne,bme->bnm')

Sharding: data-parallel over batch — core i computes batch element i:
         C = A @ B^T with A = mat_0[i], B = mat_1[i].

Per-core kernel structure (Tile framework):
  - Both operands need the contraction dim (e) on SBUF partitions, so both
    are transposed on-chip with PE transpose-mode matmuls (fp32 DMA
    transpose does not exist on TRN2).
  - B^T is built once and cached in SBUF ([128, 8, 2048] = 64KB/partition).
  - For each 128-row block of A: transpose to A^T chunks, then accumulate
    over the 8 e-chunks into [128, 512] PSUM tiles, copy back and DMA out.
"""

import sys

if "/opt/trn_rl_repo" not in sys.path:
    sys.path.insert(0, "/opt/trn_rl_repo")

import numpy as np

import concourse.bass as bass  # noqa: E402
import concourse.mybir as mybir  # noqa: E402
import concourse.tile as tile  # noqa: E402
from concourse import bacc  # noqa: E402
from concourse.bass_utils import run_bass_kernel_spmd  # noqa: E402
from concourse.masks import make_identity  # noqa: E402

P = 128

# Hardcoded problem shape (per spec nn_AttentionMatrix_41841571398230)
B_FULL, N_FULL, M_FULL, E_FULL = 8, 2048, 2048, 1024


def qkt_kernel(tc, a, b, c, n, m, e, mm_mode="f32", mg=512):
    """Emit per-core C[n, m] = A[n, :] @ B[m, :]^T.

    mm_mode: 'f32' | 'f32r' | 'fp16' | 'bf16'
      f32   — everything fp32 (4 cyc/row matmul, exact)
      f32r  — fp32 data, matmuls run with float32r bitcast (1 cyc/row @ N>=256)
      fp16  — transpose copyback casts to fp16; matmuls fp16 (1 cyc/row)
      bf16  — same with bf16
    """
    nc = tc.nc
    f32 = mybir.dt.float32
    op_dtype = {
        "f32": f32,
        "f32r": mybir.dt.float32r,
        "fp16": mybir.dt.float16,
        "bf16": mybir.dt.bfloat16,
    }[mm_mode]
    mg = min(mg, m)
    n_blocks = n // P
    m_blocks = m // P
    e_chunks = e // P
    m_groups = m // mg

    with (
        tc.tile_pool(name="const", bufs=1) as const_pool,
        tc.tile_pool(name="stage", bufs=3) as stage_pool,
        tc.tile_pool(name="tpsum", bufs=2, space="PSUM") as tpsum_pool,
        tc.tile_pool(name="btp", bufs=1) as bt_pool,
        tc.tile_pool(name="atp", bufs=2) as at_pool,
        tc.tile_pool(name="mpsum", bufs=4, space="PSUM") as mpsum_pool,
        tc.tile_pool(name="co", bufs=3) as co_pool,
    ):
        ident = const_pool.tile([P, P], f32)
        make_identity(nc, ident)

        # ---- Build B^T cached in SBUF: bt[p, k, m] = B[m, k*128+p] ----
        bt = bt_pool.tile([P, e_chunks, m], op_dtype)
        for mb in range(m_blocks):
            stage = stage_pool.tile([P, e], f32, tag="stage")
            nc.sync.dma_start(stage, b[mb * P : (mb + 1) * P, :])
            for k in range(e_chunks):
                pt = tpsum_pool.tile([P, P], f32, tag="tp")
                nc.tensor.transpose(pt, stage[:, k * P : (k + 1) * P], ident)
                nc.scalar.copy(bt[:, k, mb * P : (mb + 1) * P], pt)

        # ---- Main loop over 128-row blocks of A ----
        for nb in range(n_blocks):
            stage = stage_pool.tile([P, e], f32, tag="stage")
            nc.sync.dma_start(stage, a[nb * P : (nb + 1) * P, :])
            at = at_pool.tile([P, e_chunks, P], op_dtype, tag="at")
            for k in range(e_chunks):
                pt = tpsum_pool.tile([P, P], f32, tag="tp")
                nc.tensor.transpose(pt, stage[:, k * P : (k + 1) * P], ident)
                nc.scalar.copy(at[:, k, :], pt)

            for g in range(m_groups):
                ps = mpsum_pool.tile([P, mg], f32, tag="ps")
                for k in range(e_chunks):
                    nc.tensor.matmul(
                        ps,
                        at[:, k, :],
                        bt[:, k, g * mg : (g + 1) * mg],
                        start=(k == 0),
                        stop=(k == e_chunks - 1),
                    )
                ot = co_pool.tile([P, mg], f32, tag="ot")
                nc.vector.tensor_copy(ot, ps)
                nc.sync.dma_start(c[nb * P : (nb + 1) * P, g * mg : (g + 1) * mg], ot)


def build_qkt(n, m, e, mm_mode="f32", mg=512):
    nc = bacc.Bacc(None, target_bir_lowering=False)
    with tile.TileContext(nc) as tc:
        with tc.tile_pool(name="dram", bufs=1, space="DRAM") as dram:
            a = dram.tile([n, e], mybir.dt.float32, kind="ExternalInput", name="mat_0")
            b = dram.tile([m, e], mybir.dt.float32, kind="ExternalInput", name="mat_1")
            c = dram.tile([n, m], mybir.dt.float32, kind="ExternalOutput", name="out")
            qkt_kernel(tc, a[:], b[:], c[:], n, m, e, mm_mode=mm_mode, mg=mg)
    nc.compile()
    return nc, a.name, b.name, c.name


_CACHE = {}


def _get_built(n, m, e, mm_mode, mg=512):
    key = (n, m, e, mm_mode, mg)
    if key not in _CACHE:
        _CACHE[key] = build_qkt(n, m, e, mm_mode=mm_mode, mg=mg)
    return _CACHE[key]


def run_qkt(mat_0, mat_1, mm_mode="f32", mg=512, trace=False):
    """Run the sharded kernel on full inputs [b, n, e], [b, m, e]."""
    bsz, n, e = mat_0.shape
    _, m, _ = mat_1.shape
    nc, a_name, b_name, c_name = _get_built(n, m, e, mm_mode, mg)
    in_maps = [
        {
            a_name: np.ascontiguousarray(mat_0[i], dtype=np.float32),
            b_name: np.ascontiguousarray(mat_1[i], dtype=np.float32),
        }
        for i in range(bsz)
    ]
    res = run_bass_kernel_spmd(nc, in_maps, core_ids=list(range(bsz)), trace=trace)
    out = np.stack([res.results[i][c_name] for i in range(bsz)], axis=0)
    return out, res


def kernel(mat_0, mat_1):
    out, _ = run_qkt(
        np.asarray(mat_0, dtype=np.float32),
        np.asarray(mat_1, dtype=np.float32),
        mm_mode="f32",
    )
    return out
